# revision 1
# baseline (speedup 1.0000x reference)
"""Trainium2 Bass kernel for nn_CenterAttention.

Math (per batch b, derived from the reference):
  - x_center broadcasts x[b, 32, :] to all 64 query rows -> every row of the
    spatial output (and of the final output) is identical.
  - q = x[b,32,:] @ Wq  (one 512-vector); K = x_b @ Wk; scores s[h,m] = SCALE *
    <q_h, K[m, block_h]>; attn = softmax_m(s); P = attn @ x_b (8,512);
    o[block_h] = P[h,:] @ Wv[:, block_h]; so = o @ Wout + bout (512,)
  - spectral: S = (A2^T x2)^T-style contraction with A = Wqs @ Wks^T (64x64);
    E = exp(SCALE*S); Z = row sums; r = (so/Z) @ E (512,)
  - out[b, n, :] = r for all n.

Sharding: pure data parallel, 32 batches per core across 8 cores.
Matmul inputs in bf16 (FWL + warm PE); accumulation fp32 in PSUM.
"""

import numpy as np
import ml_dtypes
from contextlib import ExitStack

import concourse.bass as bass
import concourse.tile as tile
from concourse import bacc, mybir
from concourse.bass_utils import run_bass_kernel_spmd

B, N, D = 256, 64, 512
H, DH = 8, 64
INNER = 512
SCALE = DH ** -0.5
NCORES = 8
BC = B // NCORES          # 32 batches per core
NPAIR = BC // 2           # 16

F32 = mybir.dt.float32
BF16 = mybir.dt.bfloat16
NPBF = ml_dtypes.bfloat16

_CACHE = {}


def _build():
    nc = bacc.Bacc("TRN2", target_bir_lowering=False, debug=False,
                   num_devices=NCORES)

    dr = lambda name, shape, dt, kind="ExternalInput": nc.dram_tensor(
        name, list(shape), dt, kind=kind).ap()

    x_d = dr("x", (BC * N, D), BF16)                  # (2048, 512) row = b*64+n
    wq_d = dr("wq", (4, 128, D), BF16)
    wk_d = dr("wk", (4, 128, D), BF16)
    wv_d = dr("wv", (4, 128, D), BF16)
    wout_d = dr("wout", (4, 128, D), BF16)
    a2_d = dr("a2", (128, 128), BF16)
    selall_d = dr("selall", (BC, NPAIR * 128), BF16)
    b2sel_d = dr("b2sel", (2, 128), BF16)
    b2selt_d = dr("b2selt", (128, 2), BF16)
    halfmask_d = dr("halfmask", (128, 16), F32)
    blockmask_d = dr("blockmask", (128, D), F32)
    sel128_d = dr("sel128", (128, 16), BF16)
    bout32_d = dr("bout32", (BC, D), F32)
    ident_d = dr("ident", (128, 128), BF16)
    out_d = dr("out", (BC, N, D), F32, kind="ExternalOutput")

    ADD = mybir.AluOpType.add
    AX = mybir.AxisListType.X
    EXP = mybir.ActivationFunctionType.Exp

    with tile.TileContext(nc) as tc, ExitStack() as top:
        cp = top.enter_context(tc.tile_pool(name="consts", bufs=1))
        wq_s = cp.tile([128, 4, D], BF16)
        wk_s = cp.tile([128, 4, D], BF16)
        wv_s = cp.tile([128, 4, D], BF16)
        wout_s = cp.tile([128, 4, D], BF16)
        for c in range(4):
            nc.sync.dma_start(wq_s[:, c, :], wq_d[c])
            nc.sync.dma_start(wk_s[:, c, :], wk_d[c])
            nc.sync.dma_start(wv_s[:, c, :], wv_d[c])
            nc.sync.dma_start(wout_s[:, c, :], wout_d[c])
        a2_s = cp.tile([128, 128], BF16)
        nc.sync.dma_start(a2_s[:], a2_d[:])
        selall_s = cp.tile([BC, NPAIR * 128], BF16)
        nc.sync.dma_start(selall_s[:], selall_d[:])
        b2sel_s = cp.tile([2, 128], BF16)
        nc.sync.dma_start(b2sel_s[:], b2sel_d[:])
        b2selt_s = cp.tile([128, 2], BF16)
        nc.sync.dma_start(b2selt_s[:], b2selt_d[:])
        halfmask_s = cp.tile([128, 16], F32)
        nc.sync.dma_start(halfmask_s[:], halfmask_d[:])
        blockmask_s = cp.tile([128, D], F32)
        nc.sync.dma_start(blockmask_s[:], blockmask_d[:])
        sel128_s = cp.tile([128, 16], BF16)
        nc.sync.dma_start(sel128_s[:], sel128_d[:])
        bout_s = cp.tile([BC, D], F32)
        nc.sync.dma_start(bout_s[:], bout32_d[:])
        ident_s = cp.tile([128, 128], BF16)
        nc.sync.dma_start(ident_s[:], ident_d[:])

        x_all = cp.tile([128, NPAIR, D], BF16)        # all 32 batches, native
        for p in range(NPAIR):
            nc.sync.dma_start(x_all[:, p, :], x_d[128 * p:128 * (p + 1), :])
        x3 = x_d.rearrange("(b n) d -> b n d", n=N)
        xc_s = cp.tile([BC, D], BF16)                 # center rows x[b,32,:]
        nc.sync.dma_start(xc_s[:], x3[:, 32, :])

        qs_s = cp.tile([BC, D], BF16)                 # SCALE * q, all batches
        s2_all = cp.tile([128, 128], F32)             # scores (m-pair, 16p*8h)
        e2_all = cp.tile([128, 128], BF16)
        attn_s = cp.tile([128, 128], F32)
        soT_s = cp.tile([128, 4, BC], BF16)           # so transposed, chunked
        z_all = cp.tile([128, BC, 4], F32)            # spectral row sums
        oflat_s = cp.tile([128, 4, BC], BF16)

        # ---------------- phase 1: q, per-pair transpose/K/scores -----------
        with ExitStack() as ph1:
            ps_q = ph1.enter_context(
                tc.tile_pool(name="psq", bufs=1, space="PSUM"))
            ps_xt = ph1.enter_context(
                tc.tile_pool(name="psxt", bufs=2, space="PSUM"))
            ps_k = ph1.enter_context(
                tc.tile_pool(name="psk", bufs=2, space="PSUM"))
            ps_qbc = ph1.enter_context(
                tc.tile_pool(name="psqbc", bufs=2, space="PSUM"))
            sb1 = ph1.enter_context(tc.tile_pool(name="sb1", bufs=3))

            # q_all = xc @ Wq via PE-transposed xc chunks (ACT evacuates PSUM)
            xct_ps = ps_q.tile([128, 4, BC], BF16, tag="xct")
            for c in range(4):
                nc.tensor.transpose(xct_ps[:, c, :],
                                    xc_s[:, 128 * c:128 * (c + 1)],
                                    ident_s[0:BC, 0:BC])
            xct_s = sb1.tile([128, 4, BC], BF16, tag="xct_s")
            nc.scalar.copy(xct_s[:], xct_ps[:])
            q_ps = ps_q.tile([BC, D], F32)
            for c in range(4):
                nc.tensor.matmul(q_ps[:], xct_s[:, c, :], wq_s[:, c, :],
                                 start=(c == 0), stop=(c == 3))
            nc.vector.tensor_scalar_mul(qs_s[:], q_ps[:], SCALE)

            for p in range(NPAIR):
                xt_ps = ps_xt.tile([128, 4, 128], BF16, tag="xt")
                for c in range(4):
                    nc.tensor.transpose(xt_ps[:, c, :],
                                        x_all[:, p, 128 * c:128 * (c + 1)],
                                        ident_s[:])
                xt_s = sb1.tile([128, 4, 128], BF16, tag="xt_s")
                nc.scalar.copy(xt_s[:], xt_ps[:])

                k_ps = ps_k.tile([128, D], F32, tag="k")
                for c in range(4):
                    nc.tensor.matmul(k_ps[:], xt_s[:, c, :], wk_s[:, c, :],
                                     start=(c == 0), stop=(c == 3))

                qbc_ps = ps_qbc.tile([128, D], F32, tag="qbc")
                nc.tensor.matmul(qbc_ps[:], selall_s[:, 128 * p:128 * (p + 1)],
                                 qs_s[:])
                qbc_s = sb1.tile([128, D], BF16, tag="qbc_s")
                nc.vector.tensor_copy(qbc_s[:], qbc_ps[:])

                smul = sb1.tile([128, H, DH], F32, tag="smul")
                nc.vector.tensor_mul(smul[:].rearrange("p h m -> p (h m)"),
                                     k_ps[:], qbc_s[:])
                nc.vector.tensor_reduce(s2_all[:, 8 * p:8 * (p + 1)], smul[:],
                                        AX, ADD)

        # ---------------- phase 2: attention softmax + so ------------------
        with ExitStack() as ph2:
            ps2 = ph2.enter_context(
                tc.tile_pool(name="ps2", bufs=1, space="PSUM"))
            ps_pt = ph2.enter_context(
                tc.tile_pool(name="pspt", bufs=1, space="PSUM"))
            ps_ovw = ph2.enter_context(
                tc.tile_pool(name="psovw", bufs=2, space="PSUM"))
            sb2 = ph2.enter_context(tc.tile_pool(name="sb2", bufs=2))

            nc.scalar.activation(e2_all[:], s2_all[:], EXP)
            z2_ps = ps2.tile([2, 128], F32, tag="z2")
            nc.tensor.matmul(z2_ps[:], b2selt_s[:], e2_all[:])
            z2r_s = sb2.tile([2, 128], BF16, tag="z2r")
            with nc.allow_low_precision(reason="softmax weights used in bf16"):
                nc.vector.reciprocal(z2r_s[:], z2_ps[:])
            zbc_ps = ps2.tile([128, 128], F32, tag="zbc")
            nc.tensor.matmul(zbc_ps[:], b2sel_s[:], z2r_s[:])
            nc.vector.tensor_mul(attn_s[:], e2_all[:], zbc_ps[:])

            oflat_ps = ps2.tile([128, 4, BC], F32, tag="oflat")
            for g in range(2):
                pt_ps = ps_pt.tile([128, 4, 128], F32, tag="pt")
                for p8 in range(8):
                    p = 8 * g + p8
                    am = sb2.tile([128, 16], BF16, tag="am")
                    nc.vector.tensor_mul(am[:, 0:8], attn_s[:, 8 * p:8 * p + 8],
                                         halfmask_s[:, 0:8])
                    nc.vector.tensor_mul(am[:, 8:16], attn_s[:, 8 * p:8 * p + 8],
                                         halfmask_s[:, 8:16])
                    for c in range(4):
                        nc.tensor.matmul(
                            pt_ps[:, c, 16 * p8:16 * (p8 + 1)],
                            x_all[:, p, 128 * c:128 * (c + 1)], am[:])
                pt_s = sb2.tile([128, 4, 128], BF16, tag="pt_s")
                nc.vector.tensor_copy(pt_s[:], pt_ps[:])
                ovw_ps = ps_ovw.tile([128, D], F32, tag="ovw")
                for c in range(4):
                    nc.tensor.matmul(ovw_ps[:], pt_s[:, c, :], wv_s[:, c, :],
                                     start=(c == 0), stop=(c == 3))
                oexp_s = sb2.tile([128, D], BF16, tag="oexp")
                nc.vector.tensor_mul(oexp_s[:], ovw_ps[:], blockmask_s[:])
                for c in range(4):
                    nc.tensor.matmul(oflat_ps[:, c, 16 * g:16 * (g + 1)],
                                     oexp_s[:, 128 * c:128 * (c + 1)],
                                     sel128_s[:])
            nc.vector.tensor_copy(oflat_s[:], oflat_ps[:])

            so_ps = ps2.tile([BC, D], F32, tag="so")
            for c in range(4):
                nc.tensor.matmul(so_ps[:], oflat_s[:, c, :], wout_s[:, c, :],
                                 start=(c == 0), stop=(c == 3))
            so_s = sb2.tile([BC, D], BF16, tag="so_s")
            nc.vector.tensor_add(so_s[:], so_ps[:], bout_s[:])
            soT_ps = ps2.tile([128, 4, BC], BF16, tag="soT")
            for c in range(4):
                nc.tensor.transpose(soT_ps[:, c, :],
                                    so_s[:, 128 * c:128 * (c + 1)],
                                    ident_s[0:BC, 0:BC])
            nc.vector.tensor_copy(soT_s[:], soT_ps[:])

        # ---------------- phase 3: spectral + final -------------------------
        with ExitStack() as ph3:
            ps_g = ph3.enter_context(
                tc.tile_pool(name="psg", bufs=2, space="PSUM"))
            ps_s = ph3.enter_context(
                tc.tile_pool(name="pss", bufs=2, space="PSUM"))
            ps_o = ph3.enter_context(
                tc.tile_pool(name="pso", bufs=2, space="PSUM"))
            sb3 = ph3.enter_context(tc.tile_pool(name="sb3", bufs=2))
            sbe = ph3.enter_context(tc.tile_pool(name="sbe", bufs=3))

            out_flat = out_d.rearrange("b n d -> (b n) d")
            for p in range(NPAIR):
                g2_ps = ps_g.tile([128, D], F32, tag="g2")
                nc.tensor.matmul(g2_ps[:], a2_s[:], x_all[:, p, :])
                g2_s = sb3.tile([128, D], BF16, tag="g2s")
                nc.vector.tensor_copy(g2_s[:], g2_ps[:])
                o_ps = ps_o.tile([128, D], F32, tag="o")
                e_t = [sbe.tile([128, 2, 2, D], BF16, tag=f"e{hb}",
                                name=f"et{hb}") for hb in range(2)]
                # S matmuls for both batches interleaved: K=64 row-pairs on
                # PE tiles (0,0)/(64,0) run concurrently in 64-row mode.
                for sh in range(2):
                    s2p = [ps_s.tile([128, 2, D], F32, tag="sps",
                                    name=f"sps{i}") for i in range(2)]
                    for c2 in range(2):
                        c = 2 * sh + c2
                        for hb in range(2):
                            lo, hi = 64 * hb, 64 * (hb + 1)
                            nc.tensor.matmul(
                                s2p[hb][:, c2, :],
                                g2_s[lo:hi, 128 * c:128 * (c + 1)],
                                x_all[lo:hi, p, :],
                                tile_position=(64 * hb, 0))
                    for hb in range(2):
                        b = 2 * p + hb
                        for c2 in range(2):
                            nc.scalar.activation(
                                e_t[hb][:, sh, c2, :], s2p[hb][:, c2, :],
                                EXP, scale=SCALE,
                                accum_out=z_all[:, b, 2 * sh + c2:
                                                2 * sh + c2 + 1])
                wreps = []
                for hb in range(2):
                    b = 2 * p + hb
                    zr_b = sb3.tile([128, 4], F32, tag="zr")
                    nc.vector.reciprocal(zr_b[:], z_all[:, b, :])
                    w4_b = sb3.tile([128, 4], BF16, tag="w4")
                    nc.vector.tensor_mul(w4_b[:], soT_s[:, :, b], zr_b[:])
                    wrep = sbe.tile([128, 4, 64], BF16, tag=f"wrep{hb}")
                    nc.vector.tensor_copy(
                        wrep[:],
                        w4_b[:].rearrange("p (c u) -> p c u", u=1).broadcast_to(
                            (128, 4, 64)))
                    wreps.append(wrep)
                # final matmuls: M=64 col-pairs on PE tiles (0,0)/(0,64).
                for c in range(4):
                    for hb in range(2):
                        lo, hi = 64 * hb, 64 * (hb + 1)
                        nc.tensor.matmul(o_ps[lo:hi, :], wreps[hb][:, c, :],
                                         e_t[hb][:, c // 2, c % 2, :],
                                         start=(c == 0), stop=(c == 3),
                                         tile_position=(0, 64 * hb))
                o_s = sb3.tile([128, D], F32, tag="o_s")
                nc.vector.tensor_copy(o_s[:], o_ps[:])
                nc.sync.dma_start(out_flat[128 * p:128 * (p + 1), :], o_s[:])

    nc.compile()
    return nc


def _consts():
    c = {}
    b2 = np.zeros((2, 128), np.float32)
    for i in range(2):
        b2[i, 64 * i:64 * (i + 1)] = 1.0
    c["b2sel"] = b2.astype(NPBF)
    c["b2selt"] = np.ascontiguousarray(b2.T).astype(NPBF)
    hm = np.zeros((128, 16), np.float32)
    for j in range(16):
        hm[64 * (j // 8):64 * (j // 8 + 1), j] = 1.0
    c["halfmask"] = hm
    bm = np.zeros((128, 512), np.float32)
    for r in range(128):
        h = r % 8
        bm[r, 64 * h:64 * (h + 1)] = 1.0
    c["blockmask"] = bm
    sel = np.zeros((128, 16), np.float32)
    for r in range(128):
        sel[r, r // 8] = 1.0
    c["sel128"] = sel.astype(NPBF)
    sa = np.zeros((BC, NPAIR * 128), np.float32)
    for p in range(NPAIR):
        for m in range(128):
            sa[2 * p + m // 64, 128 * p + m] = 1.0
    c["selall"] = sa.astype(NPBF)
    c["ident"] = np.eye(128, dtype=np.float32).astype(NPBF)
    return c


def kernel(x, Wq, Wkv, Wout, bout, Wspec):
    x = np.asarray(x, np.float32)
    Wq = np.asarray(Wq, np.float32)
    Wkv = np.asarray(Wkv, np.float32)
    Wout = np.asarray(Wout, np.float32)
    bout = np.asarray(bout, np.float32)
    Wspec = np.asarray(Wspec, np.float32)

    if "nc" not in _CACHE:
        _CACHE["nc"] = _build()
        _CACHE["consts"] = _consts()
    nc = _CACHE["nc"]
    cc = _CACHE["consts"]

    A = Wspec[:, :N] @ Wspec[:, N:2 * N].T            # (64, 64)
    a2 = np.zeros((128, 128), np.float32)
    a2[:64, :64] = A
    a2[64:, 64:] = A

    base = {
        "wq": np.ascontiguousarray(Wq.reshape(4, 128, D)).astype(NPBF),
        "wk": np.ascontiguousarray(
            Wkv[:, :INNER].reshape(4, 128, D)).astype(NPBF),
        "wv": np.ascontiguousarray(
            Wkv[:, INNER:].reshape(4, 128, D)).astype(NPBF),
        "wout": np.ascontiguousarray(Wout.reshape(4, 128, D)).astype(NPBF),
        "a2": a2.astype(NPBF),
        "bout32": np.tile(bout[None, :], (BC, 1)).astype(np.float32),
        **cc,
    }
    in_maps = []
    for core in range(NCORES):
        m = dict(base)
        m["x"] = np.ascontiguousarray(
            x[BC * core:BC * (core + 1)].reshape(BC * N, D)).astype(NPBF)
        in_maps.append(m)

    _CACHE["in_maps"] = in_maps
    res = run_bass_kernel_spmd(nc, in_maps, list(range(NCORES)))
    out = np.concatenate([res.results[i]["out"] for i in range(NCORES)], axis=0)
    return out.astype(np.float32)

